# revision 4
# baseline (speedup 1.0000x reference)
"""Multi-head attention (B=2, L=2048, D=2048, H=16, Dh=128) on 8 NeuronCores.

Sharding: tensor-parallel over heads (2 heads/core) for QKV projection +
attention (dispatch A), then sequence-parallel final projection (dispatch B,
512 rows of B*L per core). Host does the small reshuffle between dispatches.

Layout strategy (per core, dispatch A):
  - host feeds x^T (D, B*L) so QKV matmuls contract over d on partitions and
    produce q^T/k^T (Dh on partitions) directly — the layout attention wants.
  - scores computed transposed: S^T[kk, l] (keys on partitions), softmax
    without max-subtraction (logits ~ N(0,1); shift by -3 for fp16 headroom),
    exp evicted to fp16 in (128,1024) pair-ops.
  - per key-pair, the Z (ones-vector row-sum) and PV matmuls are emitted right
    after the exp so the PE stays fed while ACT computes the next exp.
  - normalization: Z replicated across partitions with a K=1 matmul,
    reciprocal_approx_fast, multiply folded into the PV eviction; V-bias is
    added post-normalization (out^T layout makes bv per-partition).
  - matmuls in fp16 (full PE speed, 10-bit mantissa), fp32 PSUM accumulation.
"""

import os
import sys

import numpy as np

for _p in ("/opt/trn_rl_repo",):
    if _p not in sys.path:
        sys.path.insert(0, _p)

import concourse.bacc as bacc
import concourse.mybir as mybir
import concourse.tile as tile
from concourse.bass_utils import run_bass_kernel_spmd

P = 128
B, L, D = 2, 2048, 2048
BL = B * L
H, DH = 16, 128
NCORES = 8
HLOC = H // NCORES            # heads per core = 2
DT = D // P                   # d-tiles = 16
NET = 3 * HLOC                # e-tiles per core in dispatch A = 6 (q0,q1,k0,k1,v0,v1)
NLC = L // 512                # l-chunks of 512 per batch = 4
NKK = L // P                  # key tiles per batch = 16
LCB = BL // NCORES            # rows per core in dispatch B = 512

F32 = mybir.dt.float32
F16 = mybir.dt.float16
MM_DT = F16
MM_NP = np.float16
ACTF = mybir.ActivationFunctionType
EXP_SHIFT = -3.0

_programs = {}

# Results of the last kernel() call when BASS_MHA_TRACE=1 (for test harness).
last_run_info = {}


def _build_a():
    nc = bacc.Bacc(None, target_bir_lowering=False, debug=False)
    xT = nc.dram_tensor("xT", [D, BL], MM_DT, kind="ExternalInput")
    wqkvT = nc.dram_tensor("wqkvT", [D, NET * P], MM_DT, kind="ExternalInput")
    bias_qk = nc.dram_tensor("bias_qk", [4, P], F32, kind="ExternalInput")
    bias_v = nc.dram_tensor("bias_v", [HLOC, P], F32, kind="ExternalInput")
    ones16 = nc.dram_tensor("ones16", [P, 1], F16, kind="ExternalInput")
    ones16r = nc.dram_tensor("ones16r", [1, P], F16, kind="ExternalInput")
    ident16 = nc.dram_tensor("ident16", [P, P], F16, kind="ExternalInput")
    outT = nc.dram_tensor("outT", [HLOC * DH, BL], F32, kind="ExternalOutput")

    with tile.TileContext(nc) as tc:
        with (
            tc.tile_pool(name="const", bufs=1) as const,
            tc.tile_pool(name="xs", bufs=3) as xs,
            tc.tile_pool(name="qk", bufs=2) as qkp,
            tc.tile_pool(name="vt", bufs=2) as vtp,
            tc.tile_pool(name="vn", bufs=2) as vnp,
            tc.tile_pool(name="es", bufs=4) as esp,
            tc.tile_pool(name="ev", bufs=3) as evp,
            tc.tile_pool(name="ps", bufs=4, space="PSUM") as ps,
            tc.tile_pool(name="ps2", bufs=2, space="PSUM") as ps2p,
        ):
            w_sb = const.tile([P, DT, NET * P], MM_DT)
            nc.sync.dma_start(w_sb[:], wqkvT.rearrange("(t p) e -> p t e", p=P))
            bqk_sb = const.tile([P, 4], F32)
            nc.sync.dma_start(bqk_sb[:], bias_qk.rearrange("t p -> p t"))
            bv_sb = const.tile([P, HLOC], F32)
            nc.sync.dma_start(bv_sb[:], bias_v.rearrange("t p -> p t"))
            ones_l = const.tile([P, 1], F16)
            nc.sync.dma_start(ones_l[:], ones16[:])
            ones_r = const.tile([1, P], F16)
            nc.sync.dma_start(ones_r[:], ones16r[:])
            ident = const.tile([P, P], F16)
            nc.sync.dma_start(ident[:], ident16[:])
            shift = const.tile([P, 1], F32)
            nc.any.memset(shift[:], EXP_SHIFT)

            for b in range(B):
                # ---- Phase 1: QKV projection (transposed outputs) ----
                qk_sb = qkp.tile([P, 4, L], MM_DT, tag="qk")
                vT_sb = vtp.tile([P, HLOC, L], F16, tag="vt")
                for lc in range(NLC):
                    xts = []
                    for dh_half in range(2):
                        xt = xs.tile([P, DT // 2, 512], MM_DT, tag="xs",
                                     name=f"xt{dh_half}")
                        nc.sync.dma_start(
                            xt[:],
                            xT[
                                dh_half * (D // 2) : (dh_half + 1) * (D // 2),
                                b * L + lc * 512 : b * L + (lc + 1) * 512,
                            ].rearrange("(t p) l -> p t l", p=P),
                        )
                        xts.append(xt)
                    for grp in range(2):
                        pss = [
                            ps.tile([P, 512], F32, tag="ps", name=f"ps_qkv{j}")
                            for j in range(3)
                        ]
                        for dh_half in range(2):
                            for d8 in range(DT // 2):
                                d = dh_half * (DT // 2) + d8
                                for j in range(3):
                                    et = grp * 3 + j
                                    nc.tensor.matmul(
                                        pss[j][:],
                                        w_sb[:, d, et * P : (et + 1) * P],
                                        xts[dh_half][:, d8, :],
                                        start=(d == 0),
                                        stop=(d == DT - 1),
                                    )
                        lsl = slice(lc * 512, (lc + 1) * 512)
                        for j in range(3):
                            et = grp * 3 + j
                            if et < 4:
                                nc.vector.tensor_scalar_add(
                                    qk_sb[:, et, lsl], pss[j][:],
                                    bqk_sb[:, et : et + 1],
                                )
                            else:
                                nc.vector.tensor_copy(
                                    vT_sb[:, et - 4, lsl], pss[j][:]
                                )

                # ---- Phase 2: attention, per local head ----
                for h in range(HLOC):
                    # transpose v^T (Dh, L) -> v natural tiles (kk, Dh)
                    v_sb = vnp.tile([P, NKK, DH], F16, tag="vn")
                    for kk in range(NKK):
                        pst = ps.tile([P, P], F16, tag="ps", name="pst")
                        nc.tensor.transpose(
                            pst[:], vT_sb[:, h, kk * P : (kk + 1) * P], ident[:]
                        )
                        nc.vector.tensor_copy(v_sb[:, kk, :], pst[:])

                    for lc in range(NLC):
                        lsl = slice(lc * 512, (lc + 1) * 512)
                        ps_z = ps.tile([1, 512], F32, tag="ps", name="ps_z")
                        ps_pv = ps.tile([P, 512], F32, tag="ps", name="ps_pv")
                        for kkp in range(NKK // 2):
                            ps_s = ps2p.tile([P, 1024], F32, tag="ps2", name="ps_s")
                            es = esp.tile([P, 2, 512], F16, tag="es", name="es")
                            for half in range(2):
                                kk = 2 * kkp + half
                                nc.tensor.matmul(
                                    ps_s[:, half * 512 : (half + 1) * 512],
                                    qk_sb[:, 2 + h, kk * P : (kk + 1) * P],
                                    qk_sb[:, h, lsl],
                                    start=True,
                                    stop=True,
                                )
                            nc.scalar.activation(
                                es[:].rearrange("p a b -> p (a b)"),
                                ps_s[:],
                                ACTF.Exp,
                                bias=shift[:],
                            )
                            for half in range(2):
                                kk = 2 * kkp + half
                                nc.tensor.matmul(
                                    ps_z[:],
                                    ones_l[:],
                                    es[:, half, :],
                                    start=(kk == 0),
                                    stop=(kk == NKK - 1),
                                )
                                nc.tensor.matmul(
                                    ps_pv[:],
                                    v_sb[:, kk, :],
                                    es[:, half, :],
                                    start=(kk == 0),
                                    stop=(kk == NKK - 1),
                                )
                        # replicate Z across partitions, approx-reciprocal,
                        # normalize + V bias, store out^T chunk
                        z16 = evp.tile([1, 512], F16, tag="z16")
                        nc.vector.tensor_copy(z16[:], ps_z[:])
                        ps_zb = ps.tile([P, 512], F32, tag="ps", name="ps_zb")
                        nc.tensor.matmul(
                            ps_zb[:], ones_r[:], z16[:], start=True, stop=True
                        )
                        zb_sb = evp.tile([P, 512], F32, tag="zb")
                        nc.vector.tensor_copy(zb_sb[:], ps_zb[:])
                        recip = evp.tile([P, 512], F32, tag="recip")
                        nc.vector.reciprocal_approx_fast(recip[:], zb_sb[:])
                        out_sb = evp.tile([P, 512], F32, tag="out")
                        nc.vector.tensor_tensor(
                            out_sb[:], ps_pv[:], recip[:], mybir.AluOpType.mult
                        )
                        nc.vector.tensor_scalar_add(
                            out_sb[:], out_sb[:], bv_sb[:, h : h + 1]
                        )
                        nc.sync.dma_start(
                            outT[
                                h * DH : (h + 1) * DH,
                                b * L + lc * 512 : b * L + (lc + 1) * 512,
                            ],
                            out_sb[:],
                        )
    nc.compile()
    return nc


def _build_b():
    nc = bacc.Bacc(None, target_bir_lowering=False, debug=False)
    outTc = nc.dram_tensor("outTc", [D, LCB], MM_DT, kind="ExternalInput")
    projWT = nc.dram_tensor("projWT", [D, D], MM_DT, kind="ExternalInput")
    bias_pb = nc.dram_tensor("bias_pb", [P, D], F32, kind="ExternalInput")
    final = nc.dram_tensor("final", [LCB, D], F32, kind="ExternalOutput")

    with tile.TileContext(nc) as tc:
        with (
            tc.tile_pool(name="const", bufs=1) as const,
            tc.tile_pool(name="fo", bufs=4) as fo,
            tc.tile_pool(name="ps", bufs=6, space="PSUM") as ps,
        ):
            # full proj_w^T resident: (d-part, d-tile, e)
            pw_sb = const.tile([P, DT, D], MM_DT)
            nc.sync.dma_start(pw_sb[:], projWT.rearrange("(t p) e -> p t e", p=P))
            # out^T chunk resident: (d-part, d-tile, l)
            oc_sb = const.tile([P, DT, LCB], MM_DT)
            nc.sync.dma_start(oc_sb[:], outTc.rearrange("(t p) l -> p t l", p=P))
            # proj bias replicated across partitions (host-fed)
            pb_sb = const.tile([P, D], F32)
            nc.sync.dma_start(pb_sb[:], bias_pb[:])

            for lt in range(LCB // P):      # 4 row-tiles of 128
                pss = [
                    ps.tile([P, 512], F32, tag="ps", name=f"ps_f{ec}")
                    for ec in range(4)
                ]
                for d in range(DT):
                    # stationary: out^T (d, l-tile) reused across 4 e-chunks
                    for ec in range(4):
                        nc.tensor.matmul(
                            pss[ec][:],
                            oc_sb[:, d, lt * P : (lt + 1) * P],
                            pw_sb[:, d, ec * 512 : (ec + 1) * 512],
                            start=(d == 0),
                            stop=(d == DT - 1),
                        )
                for ec in range(4):
                    f_sb = fo.tile([P, 512], F32, tag="f")
                    nc.vector.tensor_tensor(
                        f_sb[:],
                        pss[ec][:],
                        pb_sb[:, ec * 512 : (ec + 1) * 512],
                        mybir.AluOpType.add,
                    )
                    nc.sync.dma_start(
                        final[lt * P : (lt + 1) * P, ec * 512 : (ec + 1) * 512],
                        f_sb[:],
                    )
    nc.compile()
    return nc


def _get_programs():
    if "a" not in _programs:
        _programs["a"] = _build_a()
        _programs["b"] = _build_b()
    return _programs["a"], _programs["b"]


def kernel(x, Wqkv_w, Wqkv_b, proj_w, proj_b):
    x = np.ascontiguousarray(np.asarray(x, dtype=np.float32))
    Wqkv_w = np.asarray(Wqkv_w, dtype=np.float32)
    Wqkv_b = np.asarray(Wqkv_b, dtype=np.float32)
    proj_w = np.asarray(proj_w, dtype=np.float32)
    proj_b = np.asarray(proj_b, dtype=np.float32)

    nc_a, nc_b = _get_programs()
    trace = bool(int(os.environ.get("BASS_MHA_TRACE", "0")))
    qscale = np.float32(1.0 / np.sqrt(DH))

    xT = np.ascontiguousarray(x.reshape(BL, D).T).astype(MM_NP)
    ones16 = np.ones((P, 1), np.float16)
    ones16r = np.ones((1, P), np.float16)
    ident16 = np.eye(P, dtype=np.float16)

    in_maps_a = []
    for c in range(NCORES):
        g0 = HLOC * c
        rows = []
        biases_qk = np.empty((4, P), np.float32)
        for j in range(HLOC):
            rows.append(Wqkv_w[(g0 + j) * DH : (g0 + j + 1) * DH] * qscale)
            biases_qk[j] = Wqkv_b[(g0 + j) * DH : (g0 + j + 1) * DH] * qscale
        for j in range(HLOC):
            rows.append(Wqkv_w[D + (g0 + j) * DH : D + (g0 + j + 1) * DH])
            biases_qk[HLOC + j] = Wqkv_b[D + (g0 + j) * DH : D + (g0 + j + 1) * DH]
        bias_v = np.empty((HLOC, P), np.float32)
        for j in range(HLOC):
            rows.append(Wqkv_w[2 * D + (g0 + j) * DH : 2 * D + (g0 + j + 1) * DH])
            bias_v[j] = Wqkv_b[2 * D + (g0 + j) * DH : 2 * D + (g0 + j + 1) * DH]
        wqkvT = np.ascontiguousarray(np.concatenate(rows, axis=0).T).astype(MM_NP)
        in_maps_a.append(
            {
                "xT": xT,
                "wqkvT": wqkvT,
                "bias_qk": biases_qk,
                "bias_v": bias_v,
                "ones16": ones16,
                "ones16r": ones16r,
                "ident16": ident16,
            }
        )

    res_a = run_bass_kernel_spmd(nc_a, in_maps_a, list(range(NCORES)), trace=trace)
    outT_full = np.concatenate(
        [res_a.results[c]["outT"] for c in range(NCORES)], axis=0
    )  # (D, BL)

    projWT = np.ascontiguousarray(proj_w.T).astype(MM_NP)
    bias_pb = np.ascontiguousarray(
        np.broadcast_to(proj_b[None, :], (P, D))
    ).astype(np.float32)
    in_maps_b = [
        {
            "outTc": np.ascontiguousarray(
                outT_full[:, c * LCB : (c + 1) * LCB]
            ).astype(MM_NP),
            "projWT": projWT,
            "bias_pb": bias_pb,
        }
        for c in range(NCORES)
    ]
    res_b = run_bass_kernel_spmd(nc_b, in_maps_b, list(range(NCORES)), trace=trace)
    final = np.concatenate(
        [res_b.results[c]["final"] for c in range(NCORES)], axis=0
    )  # (BL, D)

    if trace:
        last_run_info["a"] = res_a
        last_run_info["b"] = res_b

    return final.reshape(B, L, D)
